# revision 42
# baseline (speedup 1.0000x reference)
"""Trainium2 Bass kernel for a conditional GRU decoder.

Model (per reference):
  h0 = [z, x_cond] @ W_lh.T + b_lh ; x0 = 0
  for t in 0..127: hn = GRUCell(x_t, h_t); logits_t = hn @ W_out.T + b_out
Because x_{t+1} == h_{t+1}, the two GRU matmuls fuse into one (B,R)@(R,4R)
matmul with rows [Wi_r+Wh_r; Wi_z+Wh_z; Wi_n; Wh_n].

Sharding: data-parallel over batch, B=2048 -> 8 cores x 256; weights
replicated. Feature dim on partitions, batch on the free dim, so gate
biases are per-partition scalars and weights stay stationary.

Key performance structure (measured 529us; 3933ns/step steady state):
 - HAM warmup: ~3.4us of cold back-to-back MMs at start fires the PE
   clock-gate SHORT window (1.2 -> 2.4 GHz); 4 fat keepalive MMs per step
   (consuming chain tiles so they spread over the EW tail) keep every MID
   window's duty cycle high enough that the PE never re-throttles. Thin
   N=128 keepalives are NOT enough - HAM watches duty, not any-activity.
 - Per-step critical cycle: h1 -> gr-k1 MM -> sigmoid(r0) -> STT t2 ->
   idMM -> tanh0 -> tanh1 -> p1n1 -> sub1 -> h1'. ACT is ~saturated
   in-loop (r0,r1,u,tanh0,tanh1 = 2.57us of the 3.93us period); each
   PE-completion->consumer semaphore wake costs ~250ns (2 on the cycle).
 - Gate-major MM order (r gates first, z last); K=2 selector bias-MM
   injects b_z into the gz bank so u is a single [128,512] ACT; b_r/b_hn
   ride the STT scalar column; b_in/b_out ride the free ACT bias slot.
 - n-gate pre-sum folded in PSUM: DVE computes t2=(b_hn+ghn)*r, an
   identity-weight MM accumulates it onto the per-chunk gin banks
   (in-gates + idMM interleaved per chunk in the PE FIFO), tanh reads
   the bank with bias=b_in. Logits MMs from the previous step fill the
   PE gap before the in-gates.
 - Combine: p2=u*h and um1=u-1 run in the DVE gap after the t2s (f16
   SBUF fast modes); the post-tanh tail is two 2x-mode TTs per chunk
   (p1n=um1*nt, h'=p2-p1n), chunk-staggered so next-step k0 MMs start
   as soon as h chunk0 lands (scheduler orders p1n0,p1n1,sub0,sub1 -
   h1 is the carried dependency, protect it).
 - Logits identity+bias ACT packed 2 steps per instruction (one [64,
   2,256] PSUM bank), DMA'd every 2 steps; emitted after tanh-1 so it
   only uses ACT idle time. All consts in 1 packed DMA + 5 weight DMAs
   (contiguous [p,k,m] host layouts); dummy sigmoid preloads the ACT
   table set during the weight DMAs.
Matmul operands f16 (end-to-end rel err ~8e-4); PSUM accumulation f32.
"""

import numpy as np

import concourse.bass as bass
import concourse.tile as tile
from concourse import bacc, mybir
from concourse.bass_utils import run_bass_kernel_spmd

F32 = mybir.dt.float32
F16 = mybir.dt.float16
ACT = mybir.ActivationFunctionType
ALU = mybir.AluOpType

B = 2048
HID = 256
COND = 128
NCH = 64
MAXLEN = 128
R = 256
NCORES = 8
BC = B // NCORES  # 256 per-core batch
KT = R // 128     # 2 k-tiles over R
ZC = HID + COND   # 384
ZKT = ZC // 128   # 3 k-tiles over hid+cond


def _build(maxlen=MAXLEN, dbg=False):
    nc = bacc.Bacc("TRN2", target_bir_lowering=False, debug=False)

    # ---- DRAM I/O (per-core shapes; already in SBUF [p, k, m] layout so
    # the const DMAs are fully contiguous) ----
    d_zct = nc.dram_tensor("zct", [128, ZKT, BC], F16, kind="ExternalInput")
    d_wf = nc.dram_tensor("wft", [128, KT, 4 * R], F16, kind="ExternalInput")
    d_whh = nc.dram_tensor("whht", [128, KT, 3 * R], F16,
                           kind="ExternalInput")
    d_wlh = nc.dram_tensor("wlht", [128, ZKT, R], F16, kind="ExternalInput")
    d_wout = nc.dram_tensor("woutt", [128, KT, NCH], F16,
                            kind="ExternalInput")
    # per-partition bias columns: 0,1=b_r  2,3=b_hn  4,5=b_in (per chunk)
    # packed small f16 consts: ident | sel2 | blhl | bzl | biases(7 cols:
    # 0,1=b_r 2,3=b_hn 4,5=b_in per chunk, 6=b_out) along the free dim
    d_packc = nc.dram_tensor("packc", [128, 128 + KT * BC + 256 + 7], F16,
                             kind="ExternalInput")
    # logits packed 2 steps per DMA: [t//2, NCH, t%2, BC]
    d_out = nc.dram_tensor("out", [maxlen // 2, NCH, 2, BC], F32,
                           kind="ExternalOutput")
    d_ka = nc.dram_tensor("kadbg", [128, 8], F32, kind="ExternalOutput")
    d_dbg = {}
    if dbg:
        for nm in ("h0", "r", "u", "t2", "nt", "p2", "p1n", "hs", "gr",
                   "ghn", "gin"):
            d_dbg[nm] = nc.dram_tensor("dbg_" + nm, [128, KT, BC], F32,
                                       kind="ExternalOutput")

    with tile.TileContext(nc) as tc:
        with (
            tc.tile_pool(name="const", bufs=1) as const,
            tc.tile_pool(name="state", bufs=1) as state,
            tc.tile_pool(name="ew", bufs=2) as ew,
            tc.tile_pool(name="pgr", bufs=1, space="PSUM") as pgr,
            tc.tile_pool(name="pghn", bufs=1, space="PSUM") as pghn,
            tc.tile_pool(name="pgin", bufs=1, space="PSUM") as pgin,
            tc.tile_pool(name="pgz", bufs=1, space="PSUM") as pgz,
            tc.tile_pool(name="pl", bufs=1, space="PSUM") as pl,
            tc.tile_pool(name="ph0", bufs=1, space="PSUM") as ph0,
        ):
            # ---- load constants (smallest/earliest-needed first: the
            # packed selector operands feed the warmup MMs, then zct/wlh
            # for h0, whh for step 0, wf for steps 1+, wout at first flush)
            packc = const.tile([128, 128 + KT * BC + 256 + 7], F16)
            nc.sync.dma_start(packc, d_packc[:])
            ident = packc[:, 0:128]
            sel2 = packc[0:KT, 128 : 128 + KT * BC]
            blhl = packc[0:KT, 128 + KT * BC : 128 + KT * BC + 128]
            bzl = packc[0:KT, 128 + KT * BC + 128 : 128 + KT * BC + 256]
            bia = const.tile([128, 7], F32)
            nc.vector.tensor_copy(bia, packc[:, 128 + KT * BC + 256 :])
            boutc = bia[0:NCH, 6:7]
            # dummy sigmoid: pulls the ACT sigmoid/tanh table-set load
            # (~2.7us) into the weight-DMA window instead of step 0's chain
            actwarm = ew.tile([128, 8], F16, tag="actwarm")
            nc.scalar.activation(actwarm, packc[:, 0:8], ACT.Sigmoid,
                                 bias=0.0)
            zct = const.tile([128, ZKT, BC], F16)
            nc.sync.dma_start(zct, d_zct[:])
            wlh = const.tile([128, ZKT, R], F16)
            nc.sync.dma_start(wlh, d_wlh[:])
            whh = const.tile([128, KT, 3 * R], F16)
            nc.sync.dma_start(whh, d_whh[:])
            wf = const.tile([128, KT, 4 * R], F16)
            nc.sync.dma_start(wf, d_wf[:])
            wout = const.tile([128, KT, NCH], F16)
            nc.sync.dma_start(wout, d_wout[:])

            def bcol(i):
                return bia[:, i : i + 1]

            h = state.tile([128, KT, BC], F16)  # hT, chunk c = rows c*128..

            # PE keepalive: dummy accumulating matmuls on chain tiles keep
            # the HAM clock-gate at full rate through the per-step EW gap.
            ka = ph0.tile([128, KT, BC], F32, tag="ka")
            ka_started = [False]

            def keepalive_on(rhs):
                n = rhs.free_size()
                nc.tensor.matmul(ka[:, 0, 0:n] if n <= BC else ka[:],
                                 wf[:, 0, 0:128], rhs,
                                 start=(not ka_started[0]), stop=False,
                                 skip_group_check=True)
                ka_started[0] = True

            def keepalive_thin(rhs):
                # N=128 keepalive: enough to mark PE activity in the HAM
                # window without eating PE issue bandwidth (~110ns each).
                nc.tensor.matmul(ka[:, 0, 0:128], wf[:, 0, 0:128], rhs,
                                 start=(not ka_started[0]), stop=False,
                                 skip_group_check=True)
                ka_started[0] = True

            # ---- HAM warmup: ~3.4us of back-to-back cold MMs (8 x 427ns)
            # so the SHORT window fires and the PE clock un-throttles to
            # 2.4 GHz; the per-step keepalives then keep every MID window
            # non-idle. Uses the selector operands (first DMA to land) so
            # warmup overlaps the fat weight loads.
            for _ in range(8):
                nc.tensor.matmul(ka[:], blhl, sel2,
                                 start=(not ka_started[0]), stop=False,
                                 skip_group_check=True)
                ka_started[0] = True

            # ---- h0 = W_lh @ zcT + b_lh (selector bias-MM + 3 k-tiles) ----
            # (borrows the gz pool's bank; freed before step 0 writes gz)
            ph = pgz.tile([128, KT, BC], F32, tag="gz")
            nc.tensor.matmul(ph[:], blhl, sel2,
                             start=True, stop=False, skip_group_check=True)
            for m in range(KT):
                for k in range(ZKT):
                    nc.tensor.matmul(ph[:, m, :], wlh[:, k, bass.ts(m, 128)],
                                     zct[:, k, :], start=False,
                                     stop=(m == KT - 1 and k == ZKT - 1),
                                     skip_group_check=True)
            nc.scalar.activation(h[:],
                                 ph[:], ACT.Identity,
                                 bias=0.0)
            dbg_f32 = {}
            if dbg:
                for nm in d_dbg:
                    dbg_f32[nm] = const.tile([128, KT, BC], F32,
                                             name="dbgt_" + nm)
                nc.vector.tensor_copy(dbg_f32["h0"], h[:])
                nc.sync.dma_start(d_dbg["h0"][:], dbg_f32["h0"])

            # pending logits emitters from the previous step; the MM half
            # flushes early (fills the PE gap before idMM-1), the
            # identity+DMA half (packed 2 steps per instr) flushes after
            # tanh-1 so it never delays the chain on ACT.
            pending = []
            pending_io = []
            lp_hold = [None]  # lp tile persisting across a 2-step pair

            def flush_logits_mm():
                for fn in pending:
                    fn()
                pending.clear()

            def flush_logits_io():
                for fn in pending_io:
                    fn()
                pending_io.clear()

            def emit_step(t, first):
                if first:
                    # x=0: gates come from W_hh only (pytorch order r,z,n)
                    w, offs = whh, {"r": 0, "z": R, "hn": 2 * R}
                else:
                    w, offs = wf, {"r": 0, "z": R, "in": 2 * R, "hn": 3 * R}

                gr = pgr.tile([128, KT, BC], F32, tag="gr")
                ghn = pghn.tile([128, KT, BC], F32, tag="ghn")
                gin0 = pgin.tile([128, BC], F32, tag="gin0")
                gin1 = pgin.tile([128, BC], F32, tag="gin1")
                ginm = (gin0, gin1)
                gz = pgz.tile([128, KT, BC], F32, tag="gz")

                def gate_mm(dst, name, m, k, start, stop):
                    d = dst[m] if isinstance(dst, tuple) else dst[:, m, :]
                    nc.tensor.matmul(
                        d, w[:, k, bass.ds(offs[name] + m * 128, 128)],
                        h[:, k, :], start=start, stop=stop,
                        skip_group_check=True)

                # ---- PE burst (gate-major: r completes first so the
                # chain-head sigmoids unblock ~700ns earlier) ----
                # gz bias-MM first: no h dependency, runs during prev EW tail
                nc.tensor.matmul(gz[:], bzl, sel2,
                                 start=True, stop=False, skip_group_check=True)
                # per-m accumulation groups (stop at each m's k1): the r0
                # sigmoid then waits only the two gr-m0 MMs, not the whole
                # r group - starts ~220ns earlier (same for t2's ghn-m0)
                for name in ("r", "hn", "z"):
                    dst = {"r": gr, "hn": ghn, "z": gz}[name]
                    for m in range(KT):
                        for k in range(KT):
                            gate_mm(dst, name, m, k,
                                    start=(name in ("r", "hn") and m == 0
                                           and k == 0),
                                    stop=(k == KT - 1))
                # prev step's logits MMs fill the PE queue here; the
                # in-gates are emitted per-chunk inside the m-loop below so
                # idMM-0 runs right after in-m0 instead of all four in-gates
                flush_logits_mm()

                # ---- ACT: r per chunk (chain head), u merged ----
                r = ew.tile([128, KT, BC], F16, tag="r")
                with tc.high_priority(64):
                    for m in range(KT):
                        nc.scalar.activation(r[:, m, :], gr[:, m, :],
                                             ACT.Sigmoid, bias=bcol(0 + m))
                # ---- DVE: t2 = (b_hn + ghn) * r, then identity-MMs fold
                # t2 into the gin tiles (pre-sum in PSUM), tanh reads them
                # with b_in in the free ACT bias slot.
                u = ew.tile([128, KT, BC], F16, tag="u")
                nc.scalar.activation(u[:], gz[:], ACT.Sigmoid, bias=0.0)
                t2 = ew.tile([128, KT, BC], F16, tag="t2")
                nt = ew.tile([128, KT, BC], F16, tag="nt")
                for m in range(KT):
                    if not first:
                        for k in range(KT):
                            gate_mm(ginm, "in", m, k,
                                    start=(k == 0), stop=False)
                    nc.vector.scalar_tensor_tensor(
                        t2[:, m, :], ghn[:, m, :], bcol(2 + m), r[:, m, :],
                        op0=ALU.add, op1=ALU.mult)
                    nc.tensor.matmul(ginm[m], ident, t2[:, m, :],
                                     start=first, stop=True,
                                     skip_group_check=True)
                    nc.scalar.activation(nt[:, m, :], ginm[m], ACT.Tanh,
                                         bias=bcol(4 + m))
                # prev step's packed logits identity+DMA: after tanh-1 in
                # the ACT FIFO so it never blocks the chain
                flush_logits_io()

                # ---- combine: p2 = u*h, p1n = (u-1)*n, h' = p2 - p1n ----
                # p2 and um1=(u-1) on the DVE idle gap after the t2s: all
                # f16 SBUF ops (fast modes), so the post-tanh tail is two
                # 2x-mode TTs per chunk instead of 1x STTs.
                p1n = ew.tile([128, KT, BC], F16, tag="p1n")
                p2 = ew.tile([128, KT, BC], F16, tag="p2")
                um1 = ew.tile([128, KT, BC], F16, tag="um1")
                nc.vector.tensor_scalar_add(um1[:], u[:], -1.0)
                nc.vector.tensor_mul(p2[:, 0, :], u[:, 0, :], h[:, 0, :])
                with tc.high_priority(512):
                    nc.vector.tensor_mul(p1n[:, 0, :], um1[:, 0, :],
                                         nt[:, 0, :])
                    nc.vector.tensor_sub(h[:, 0, :], p2[:, 0, :],
                                         p1n[:, 0, :])
                # p2-1 emitted after the chunk-0 combine: not needed until
                # sub1, so it must not block p1n0 in the DVE queue
                nc.vector.tensor_mul(p2[:, 1, :], u[:, 1, :], h[:, 1, :])
                with tc.high_priority(256):
                    nc.vector.tensor_mul(p1n[:, 1, :], um1[:, 1, :],
                                         nt[:, 1, :])
                    nc.vector.tensor_sub(h[:, 1, :], p2[:, 1, :],
                                         p1n[:, 1, :])
                if dbg and t == 0:
                    for nm, tl in (("r", r), ("u", u), ("t2", t2),
                                   ("nt", nt), ("p2", p2),
                                   ("p1n", p1n), ("hs", h)):
                        nc.vector.tensor_copy(dbg_f32[nm], tl[:])
                        nc.sync.dma_start(d_dbg[nm][:], dbg_f32[nm])

                # keepalives spread through the late EW tail (fat N=512 MMs
                # keep the PE duty cycle high enough that HAM stays warm;
                # the in-gate/logits MMs cover the early tail)
                keepalive_on(t2[:])
                keepalive_on(nt[:, 1, :])
                keepalive_on(nt[:, 0, :])
                keepalive_on(nt[:])

                # ---- logits (MMs deferred into next burst; identity+DMA
                # packed per 2 steps) ----
                def do_logits_mm(t=t):
                    if t % 2 == 0:
                        lp_hold[0] = pl.tile([NCH, 2, BC], F32, tag="lp",
                                             name="lp")
                    lp = lp_hold[0]
                    s = t % 2
                    for k in range(KT):
                        nc.tensor.matmul(lp[:, s, :], wout[:, k, :],
                                         h[:, k, :],
                                         start=(k == 0 and s == 0),
                                         stop=(k == KT - 1 and s == 1),
                                         skip_group_check=True)
                pending.append(do_logits_mm)

                def do_logits_io(t=t):
                    if t % 2 == 1:
                        lp = lp_hold[0]
                        ls = ew.tile([NCH, 2, BC], F32, tag="ls")
                        nc.scalar.activation(ls, lp[:], ACT.Identity,
                                             bias=boutc)
                        nc.sync.dma_start(d_out[t // 2], ls)
                pending_io.append(do_logits_io)

            emit_step(0, first=True)
            for t in range(1, maxlen):
                emit_step(t, first=False)
            flush_logits_mm()
            flush_logits_io()
            kcopy = ew.tile([128, 8], F32, tag="kcopy")
            nc.scalar.activation(kcopy, ka[:, 0, 0:8], ACT.Identity, bias=0.0)
            nc.sync.dma_start(d_ka[:], kcopy)

    nc.compile()
    return nc


_CACHE = {}
_LAST_IN_MAPS = None


def kernel(z, x_cond, W_lh, b_lh, W_ih, W_hh, b_ih, b_hh, W_out, b_out):
    z = np.asarray(z, np.float32)
    x_cond = np.asarray(x_cond, np.float32)
    W_lh = np.asarray(W_lh, np.float32)
    b_lh = np.asarray(b_lh, np.float32)
    W_ih = np.asarray(W_ih, np.float32)
    W_hh = np.asarray(W_hh, np.float32)
    b_ih = np.asarray(b_ih, np.float32)
    b_hh = np.asarray(b_hh, np.float32)
    W_out = np.asarray(W_out, np.float32)
    b_out = np.asarray(b_out, np.float32)

    # fused recurrent weight: rows [Wi_r+Wh_r; Wi_z+Wh_z; Wi_n; Wh_n]
    Wf = np.concatenate(
        [W_ih[:R] + W_hh[:R], W_ih[R : 2 * R] + W_hh[R : 2 * R],
         W_ih[2 * R :], W_hh[2 * R :]], axis=0)
    b_r = b_ih[:R] + b_hh[:R]
    b_z = b_ih[R : 2 * R] + b_hh[R : 2 * R]
    b_in = b_ih[2 * R :]
    b_hn = b_hh[2 * R :]

    def pcols(v):  # (R,) -> (128, KT) per-partition columns
        return np.ascontiguousarray(v.reshape(KT, 128).T)

    bout_col = np.zeros((128, 1), np.float32)
    bout_col[:NCH, 0] = b_out
    biases = np.concatenate(
        [pcols(b_r), pcols(b_hn), pcols(b_in), bout_col], axis=1)  # (128, 7)

    f16 = np.float16

    def pkm(wt, kt):  # (K, M) -> (128, kt, M) SBUF layout, contiguous
        km = wt.shape[1]
        return np.ascontiguousarray(
            wt.reshape(kt, 128, km).transpose(1, 0, 2), dtype=f16)

    wft = pkm(Wf.T, KT)            # (128, KT, 4R)
    whht = pkm(W_hh.T, KT)         # (128, KT, 3R)
    wlht = pkm(W_lh.T, ZKT)        # (128, ZKT, R)
    woutt = pkm(W_out.T, KT)       # (128, KT, NCH)
    # packed f16 consts: ident | sel2 | blhl | bzl | biases
    packc = np.zeros((128, 128 + KT * BC + 256 + 7), dtype=f16)
    packc[:, 0:128] = np.eye(128, dtype=f16)
    for c in range(KT):
        packc[c, 128 + c * BC : 128 + (c + 1) * BC] = 1.0
    packc[0:KT, 128 + KT * BC : 128 + KT * BC + 128] = \
        b_lh.reshape(KT, 128).astype(f16)
    packc[0:KT, 128 + KT * BC + 128 : 128 + KT * BC + 256] = \
        b_z.reshape(KT, 128).astype(f16)
    packc[:, 128 + KT * BC + 256 :] = biases.astype(f16)
    zct_full = np.concatenate([z, x_cond], axis=1).T.astype(f16)  # (ZC, B)

    if "nc" not in _CACHE:
        _CACHE["nc"] = _build()
    nc = _CACHE["nc"]

    in_maps = []
    for c in range(NCORES):
        in_maps.append({
            "zct": pkm(zct_full[:, c * BC : (c + 1) * BC], ZKT),
            "wft": wft,
            "whht": whht,
            "wlht": wlht,
            "woutt": woutt,
            "packc": packc,
        })

    global _LAST_IN_MAPS
    _LAST_IN_MAPS = in_maps
    res = run_bass_kernel_spmd(nc, in_maps, core_ids=list(range(NCORES)))
    # per-core out: (MAXLEN//2, NCH, 2, BC) -> (BC, MAXLEN, NCH)
    parts = [np.asarray(res.results[c]["out"]).transpose(3, 0, 2, 1)
             .reshape(BC, MAXLEN, NCH) for c in range(NCORES)]
    return np.ascontiguousarray(np.concatenate(parts, axis=0), dtype=np.float32)

